# revision 36
# baseline (speedup 1.0000x reference)
"""Multi-head attention (B=2, S=2048, D=1024, H=16) on 8 Trainium2 cores.

Sharding: core c -> (batch b = c//4, head-group g = c%4, 4 heads each).
Tensor-parallel over heads within a batch; the output projection is done
per head-group against the matching Wo column slice and the partial
[S, D] results are summed on the host (plus the folded biases bo + Wo@bv).

All on-device matmuls run in float32r (full-rate PE streaming); exp runs on
the scalar engine in 1024-wide tiles; the softmax denominator comes from a
ones-column appended to V in the PV matmul.
"""

from contextlib import ExitStack

import numpy as np

import concourse.bacc as bacc
import concourse.tile as tile
from concourse import mybir

D_MODEL = 1024
NUM_HEADS = 16
D_K = 64
B = 2
S_FULL = 2048
N_CORES = 8
GH = 4              # heads per core
GJ = GH * D_K       # 256 columns per head-group

F32 = mybir.dt.float32
F32R = mybir.dt.float32r
AF = mybir.ActivationFunctionType
ALU = mybir.AluOpType


def build_nc(S=S_FULL, SB=512):
    """Build + compile the per-core program (identical on all 8 cores)."""
    HB = min(2 * SB, S)   # sq half-width (2 blocks)
    assert S % HB == 0
    NH = S // HB      # number of sq halves
    ST = S // 128     # sk tiles
    DT = D_MODEL // 128
    JT = GJ // 128    # 2 j-tiles (2 heads each)

    nc = bacc.Bacc("TRN2", target_bir_lowering=False, debug=False)

    NCH_ = S // SB
    DT_ = D_MODEL // 128
    xqT = nc.dram_tensor("xqT", [NCH_, 128, DT_, SB], F32R, kind="ExternalInput").ap()
    xkT = nc.dram_tensor("xkT", [NCH_, 128, DT_, SB], F32R, kind="ExternalInput").ap()
    xvT = nc.dram_tensor("xvT", [NCH_, 128, DT_, SB], F32R, kind="ExternalInput").ap()
    wqT = nc.dram_tensor("wqT", [128, DT_, GJ], F32R, kind="ExternalInput").ap()
    wkT = nc.dram_tensor("wkT", [128, DT_, GJ], F32R, kind="ExternalInput").ap()
    wvT = nc.dram_tensor("wvT", [128, DT_, GJ], F32R, kind="ExternalInput").ap()
    woT = nc.dram_tensor("woT", [128, GJ // 128, D_MODEL], F32R, kind="ExternalInput").ap()
    bq = nc.dram_tensor("bq", [128, GJ // 128], F32, kind="ExternalInput").ap()
    bk = nc.dram_tensor("bk", [128, GJ // 128], F32, kind="ExternalInput").ap()
    yT = nc.dram_tensor("yT", [D_MODEL, S], F32, kind="ExternalOutput").ap()

    with tile.TileContext(nc) as tc:
        with ExitStack() as ctx:
            cpool = ctx.enter_context(tc.tile_pool(name="const", bufs=1))
            xs_pool = ctx.enter_context(tc.tile_pool(name="xs", bufs=4))
            p_pool = ctx.enter_context(tc.tile_pool(name="pt", bufs=5))
            y_pool = ctx.enter_context(tc.tile_pool(name="ys", bufs=4))
            s_pool = ctx.enter_context(tc.tile_pool(name="sm", bufs=3))
            ps_s = ctx.enter_context(tc.tile_pool(name="ps2", bufs=2, space="PSUM"))
            ps_o = ctx.enter_context(tc.tile_pool(name="po2", bufs=4, space="PSUM"))

            # ---- persistent SBUF ----
            wq_sb = cpool.tile([128, DT, GJ], F32R, tag="wq")
            wk_sb = cpool.tile([128, DT, GJ], F32R, tag="wk")
            wv_sb = cpool.tile([128, DT, GJ], F32R, tag="wv")
            wo_sb = cpool.tile([128, JT, D_MODEL], F32R, tag="wo")
            bq_sb = cpool.tile([128, JT], F32, tag="bq")
            bk_sb = cpool.tile([128, JT], F32, tag="bk")
            nc.scalar.dma_start(wk_sb[:], wkT)
            nc.scalar.dma_start(wv_sb[:], wvT)
            nc.scalar.dma_start(wq_sb[:], wqT)
            nc.scalar.dma_start(bq_sb[:], bq)
            nc.scalar.dma_start(bk_sb[:], bk)
            nc.scalar.dma_start(wo_sb[:], woT)

            qhT_sb = cpool.tile([128, JT, S], F32R, tag="qhT")
            khT_sb = cpool.tile([128, JT, S], F32R, tag="khT")
            vh_sb = cpool.tile([128, ST, GH, 65], F32R, tag="vh")
            oall_sb = cpool.tile([128, JT, S], F32R, tag="oall")

            ones_sb = cpool.tile([128, 1], F32, tag="ones")
            nc.vector.memset(ones_sb[:], 1.0)
            nc.vector.tensor_copy(
                vh_sb[:, :, :, 64:65],
                ones_sb[:, None, :].broadcast_to([128, ST, GH, 1]),
            )

            # ---- stage helpers ----
            def a_chunk(which, sb):
                """Project one 512-wide chunk of x (k/v/q) on the PE."""
                xT, w_sb, b_sb, outT, dma_eng = {
                    "k": (xkT, wk_sb, bk_sb, khT_sb, nc.sync),
                    "v": (xvT, wv_sb, None, None, nc.scalar),
                    "q": (xqT, wq_sb, bq_sb, qhT_sb, nc.sync),
                }[which]
                ss = slice(sb * SB, (sb + 1) * SB)
                xs = xs_pool.tile([128, DT, SB], F32R, tag="xs", name=f"xs_{which}{sb}")
                dma_eng.dma_start(xs[:], xT[sb])
                ps = ps_s.tile([128, JT * SB], F32, tag="ps", name=f"psa_{which}{sb}")
                if outT is not None:
                    # qhT / khT: [GJ, S] transposed projections + bias
                    for jt in range(JT):
                        sl = slice(jt * SB, (jt + 1) * SB)
                        for d in range(DT):
                            nc.tensor.matmul(
                                ps[:, sl],
                                w_sb[:, d, jt * 128:(jt + 1) * 128],
                                xs[:, d, :],
                                start=(d == 0),
                                stop=(d == DT - 1),
                            )
                        nc.vector.tensor_scalar_add(
                            outT[:, jt, ss], ps[:, sl], b_sb[:, jt:jt + 1]
                        )
                else:
                    # vh: normal layout [S, GJ], x tiles stationary
                    for stl in range(SB // 128):
                        st = sb * (SB // 128) + stl
                        sl = slice(stl * GJ, (stl + 1) * GJ)
                        for d in range(DT):
                            nc.tensor.matmul(
                                ps[:, sl],
                                xs[:, d, stl * 128:(stl + 1) * 128],
                                wv_sb[:, d, :],
                                start=(d == 0),
                                stop=(d == DT - 1),
                            )
                        nc.vector.tensor_copy(
                            vh_sb[:, st, :, 0:64],
                            ps[:, sl].rearrange("p (h e) -> p h e", h=GH),
                        )

            # Stage B: the two heads of a pair sit at partition bases 0 / 64
            # of the same j-tile, so their K=64 score matmuls land on disjoint
            # PE row groups and run concurrently; one 1024-wide exp covers both.
            po_live = {}
            po_pend = {}

            def b_pair(hf, hp, st_lo, st_hi, fillers=None):
                hs = slice(hf * SB, (hf + 1) * SB)
                jt = hp
                if st_lo == 0:
                    po_a = ps_o.tile([65, SB], F32, tag="po", name=f"po_{hf}_{hp}_a")
                    po_b = ps_o.tile([65, SB], F32, tag="po", name=f"po_{hf}_{hp}_b")
                    po_live[(hf, hp)] = [po_a, po_b]
                po_ab = po_live[(hf, hp)]

                def pv_mms(st, pt):
                    for hl in range(2):
                        nc.tensor.matmul(
                            po_ab[hl][:],
                            vh_sb[:, st, 2 * hp + hl, :],
                            pt[:, hl * SB:(hl + 1) * SB],
                            start=(st == 0),
                            stop=(st == ST - 1),
                        )

                # scores run one st ahead of PV so the PE never waits on exp
                pend = po_pend.pop((hf, hp), None)
                for st in range(st_lo, st_hi):
                    ps = ps_s.tile([128, JT * SB], F32, tag="ps",
                                   name=f"psb_{hf}_{hp}_{st}")
                    for hl in range(2):
                        base = 64 * hl
                        nc.tensor.matmul(
                            ps[:, hl * SB:(hl + 1) * SB],
                            khT_sb[base:base + 64, jt, st * 128:(st + 1) * 128],
                            qhT_sb[base:base + 64, jt, hs],
                            start=True, stop=True,
                        )
                    if pend is not None:
                        pv_mms(*pend)
                    pt = p_pool.tile([128, JT * SB], F32R, tag="pt",
                                     name=f"pt_{hf}_{hp}_{st}")
                    nc.scalar.activation(pt[:], ps[:], AF.Exp, scale=0.125)
                    pend = (st, pt)
                    if fillers and st >= st_lo + 2:
                        fillers.pop(0)()
                if st_hi == ST:
                    if pend is not None:
                        pv_mms(*pend)
                else:
                    po_pend[(hf, hp)] = pend
                if st_hi == ST:
                    for hl in range(2):
                        base = 64 * hl
                        po = po_ab[hl]
                        rcp = s_pool.tile([1, SB], F32, tag="rcp",
                                          name=f"rcp_{hf}_{hp}_{hl}")
                        nc.vector.reciprocal(rcp[:], po[64:65, :])
                        bcast = s_pool.tile([64, SB], F32, tag="bcast",
                                            name=f"bc_{hf}_{hp}_{hl}")
                        nc.gpsimd.partition_broadcast(bcast[:], rcp[:])
                        nc.vector.tensor_mul(
                            oall_sb[base:base + 64, jt, hs], po[0:64, :], bcast[:]
                        )

            def c_block(hf):
                hs = slice(hf * SB, (hf + 1) * SB)
                yr = yT.rearrange("(t p) s -> t p s", p=128)
                for mt in range(DT):
                    pc = ps_o.tile([128, SB], F32, tag="po", name=f"pc_{hf}_{mt}")
                    for kt in range(JT):
                        nc.tensor.matmul(
                            pc[:],
                            wo_sb[:, kt, mt * 128:(mt + 1) * 128],
                            oall_sb[:, kt, hs],
                            start=(kt == 0),
                            stop=(kt == JT - 1),
                        )
                    yt = y_pool.tile([128, SB], F32, tag="yt", name=f"yt_{hf}_{mt}")
                    nc.vector.tensor_copy(yt[:], pc[:])
                    (nc.sync if mt % 2 else nc.scalar).dma_start(
                        yr[mt, :, hs], yt[:]
                    )

            yr = yT.rearrange("(t p) s -> t p s", p=128)

            def c_units(hf):
                hs = slice(hf * SB, (hf + 1) * SB)
                units = []
                for mt in range(DT):
                    def u(mt=mt):
                        pc = ps_o.tile([128, SB], F32, tag="po",
                                       name=f"pc_{hf}_{mt}")
                        for kt in range(JT):
                            nc.tensor.matmul(
                                pc[:],
                                wo_sb[:, kt, mt * 128:(mt + 1) * 128],
                                oall_sb[:, kt, hs],
                                start=(kt == 0),
                                stop=(kt == JT - 1),
                            )
                        yt = y_pool.tile([128, SB], F32, tag="yt",
                                         name=f"yt_{hf}_{mt}")
                        nc.vector.tensor_copy(yt[:], pc[:])
                        (nc.sync if mt % 2 else nc.scalar).dma_start(
                            yr[mt, :, hs], yt[:]
                        )
                    units.append(u)
                return units

            def q_load(sb):
                xs = xs_pool.tile([128, DT, SB], F32R, tag="xs",
                                  name=f"xs_qf{sb}")
                nc.sync.dma_start(xs[:], xqT[sb])
                return xs

            def q_proj_units(sb, xs):
                ss = slice(sb * SB, (sb + 1) * SB)
                units = []
                for jt in range(JT):
                    def u(jt=jt, xs=xs):
                        psq = ps_o.tile([128, SB], F32, tag="po",
                                        name=f"psq_{sb}_{jt}")
                        for d in range(DT):
                            nc.tensor.matmul(
                                psq[:],
                                wq_sb[:, d, jt * 128:(jt + 1) * 128],
                                xs[:, d, :],
                                start=(d == 0),
                                stop=(d == DT - 1),
                            )
                        nc.vector.tensor_scalar_add(
                            qhT_sb[:, jt, ss], psq[:], bq_sb[:, jt:jt + 1]
                        )
                    units.append(u)
                return units

            # ---- fused schedule ----
            # hf block 0 of stage B is interleaved with stage A chunk-wise:
            # B consumes k/v sk-tiles as each chunk's projection lands.
            NCH = S // SB           # chunks
            STB = ST // NCH         # sk-tiles per chunk
            a_chunk("k", 0)
            a_chunk("v", 0)
            a_chunk("q", 0)
            q_next = None
            for sb in range(NCH):
                if sb > 0:
                    a_chunk("k", sb)
                    a_chunk("v", sb)
                if sb == NCH - 2 and NCH > 1:
                    q_next = q_load(1)
                for hp in range(GH // 2):
                    b_pair(0, hp, sb * STB, (sb + 1) * STB)
            for hf in range(1, NCH):
                qu = q_proj_units(hf, q_next)
                qu[0]()
                fillers = [qu[1]] + c_units(hf - 1)
                if hf + 1 < NCH:
                    q_next = q_load(hf + 1)
                b_pair(hf, 0, 0, ST, fillers)
                b_pair(hf, 1, 0, ST, fillers)
                for u in fillers:
                    u()
            c_block(NCH - 1)
            if NCH == 1:
                c_block(0)

    nc.compile()
    return nc


_NC_CACHE = {}


def _get_nc(S=S_FULL):
    if S not in _NC_CACHE:
        _NC_CACHE[S] = build_nc(S)
    return _NC_CACHE[S]


def make_in_maps(q, k, v, Wq, bq, Wk, bk, Wv, bv, Wo, bo, S=S_FULL):
    q = np.asarray(q, np.float32)
    k = np.asarray(k, np.float32)
    v = np.asarray(v, np.float32)
    Wq = np.asarray(Wq, np.float32)
    Wk = np.asarray(Wk, np.float32)
    Wv = np.asarray(Wv, np.float32)
    Wo = np.asarray(Wo, np.float32)
    bq = np.asarray(bq, np.float32)
    bk = np.asarray(bk, np.float32)

    SB = 512
    NCH = S // SB
    DT = D_MODEL // 128

    def xtile(x):
        # [S, D] -> xT [D, S] -> [NCH, 128, DT, SB]: t[sb, p, d, s] = x[sb*SB+s, d*128+p]
        xT = x.T  # [D, S]
        return np.ascontiguousarray(
            xT.reshape(DT, 128, NCH, SB).transpose(2, 1, 0, 3)
        )

    def wtile(wT):
        # [D, GJ] -> [128, DT, GJ]
        return np.ascontiguousarray(wT.reshape(DT, 128, GJ).transpose(1, 0, 2))

    in_maps = []
    for c in range(N_CORES):
        b, g = divmod(c, GH)
        sl = slice(g * GJ, (g + 1) * GJ)
        woT = Wo[:, sl].T  # [GJ, D]
        in_maps.append({
            "xqT": xtile(q[b, :S]),
            "xkT": xtile(k[b, :S]),
            "xvT": xtile(v[b, :S]),
            "wqT": wtile(Wq[sl].T),
            "wkT": wtile(Wk[sl].T),
            "wvT": wtile(Wv[sl].T),
            "woT": np.ascontiguousarray(
                woT.reshape(2, 128, D_MODEL).transpose(1, 0, 2)
            ),
            "bq": np.ascontiguousarray(bq[sl].reshape(2, 128).T),
            "bk": np.ascontiguousarray(bk[sl].reshape(2, 128).T),
        })
    return in_maps


def gather_out(results, Wo, bv, bo, S=S_FULL):
    Wo = np.asarray(Wo, np.float32)
    bv = np.asarray(bv, np.float32)
    bo = np.asarray(bo, np.float32)
    out = np.zeros((B, S, D_MODEL), np.float32)
    for c in range(N_CORES):
        out[c // GH] += results[c]["yT"].T
    out += bo + Wo @ bv
    return out


def kernel(q, k, v, Wq, bq, Wk, bk, Wv, bv, Wo, bo):
    from concourse.bass_utils import run_bass_kernel_spmd

    nc = _get_nc(S_FULL)
    in_maps = make_in_maps(q, k, v, Wq, bq, Wk, bk, Wv, bv, Wo, bo)
    res = run_bass_kernel_spmd(nc, in_maps, core_ids=list(range(N_CORES)))
    return gather_out(res.results, Wo, bv, bo)
